# revision 1
# baseline (speedup 1.0000x reference)
"""CapsNet forward kernel for Trainium2, 8-core data-parallel.

Strategy (per spec sharding_hint): batch (512) split across 8 cores (64 each);
all params replicated. Routing logits b are a batch-mean -> AllGather of
per-core partial deltas (1152 floats) per routing round (rounds 1,2 only;
round 3's b update is dead in the reference).

Math restructuring (keeps exact semantics, avoids materializing u):
  r := s*1152 + n  (s=caps idx, n=(c32,oy,ox))  == co*36 + pix  with co=s*32+c32
  xr2[b, r]   = primary-caps output (relu), flattened
  W2n[r, hl]  = W.transpose(3,0,1,2).reshape(9216,160)
  s[b,hl]  = sum_r c[n(r)] * W2n[r,hl] * xr2[b,r]        (matmul, K=9216)
  v        = squash_dim1(s)
  G[r,hl]  = sum_b xr2[b,r] * v[b,hl]                    (matmul, K=64/core)
  delta[n] = 1/(B*160) * sum_s sum_hl W2n[r,hl]*G[r,hl]  (DVE TT-reduce)
Convs are PE matmuls: conv1 via in-SBUF "wide patch" im2col (K=81),
primary-caps conv via 81 shifted-window matmuls accumulated in PSUM (K=256).
All big matmuls run as float32r (full-rate fp32 PE mode).
"""

import numpy as np

import concourse.bass as bass
import concourse.mybir as mybir
import concourse.tile as tile
from concourse.ap import AP
from concourse.bass_utils import run_bass_kernel_spmd

F32 = mybir.dt.float32
F32R = mybir.dt.float32r
AL = mybir.AluOpType
AF = mybir.ActivationFunctionType
AX = mybir.AxisListType

NCORES = 8
B = 512
BC = B // NCORES           # 64 images per core
MAX_WAITS = 1              # walrus on this path allows 1 sync wait per inst
HL = 160                   # 10 classes x 16 pose
NS = 9216                  # 1152 caps x 8
NT = NS // 128             # 72 K-tiles
GROUPS = [(0, 14), (14, 14), (28, 14), (42, 14), (56, 8)]  # conv2 image groups
ROUTE_SCALE = 1.0 / (B * HL)


def _r(t, dims):
    """Raw AP on tile/ap t with explicit [step, count] dims (elements)."""
    return AP(t.tensor, t.offset, dims)


def split_waits(nc, max_waits=MAX_WAITS):
    """This walrus build rejects >max_waits sync waits per instruction; move
    excess waits onto same-engine NoOps inserted immediately before."""
    for f in nc.m.functions:
        for blk in f.blocks:
            out = []
            for ins in blk.instructions:
                si = ins.sync_info
                if si is not None and si.on_wait and len(si.on_wait) > max_waits:
                    waits = list(si.on_wait)
                    k = 0
                    while len(waits) > max_waits:
                        chunk, waits = waits[:max_waits], waits[max_waits:]
                        nop = mybir.InstNoOp(name=f"{ins.name}-ws{k}", ins=[], outs=[])
                        nop.engine = ins.engine
                        nop.sync_info = mybir.SyncInfo(on_wait=chunk, on_update=[])
                        out.append(nop)
                        k += 1
                    ins.sync_info = mybir.SyncInfo(
                        on_wait=waits, on_update=list(si.on_update or []))
                out.append(ins)
            blk.instructions = out


def build_nc():
    nc = bass.Bass(num_devices=NCORES)

    xs = nc.dram_tensor("xs", [BC, 800], F32R, kind="ExternalInput")
    w1t = nc.dram_tensor("w1t", [81, 256], F32R, kind="ExternalInput")
    b1 = nc.dram_tensor("b1", [256], F32, kind="ExternalInput")
    pcwt = nc.dram_tensor("pcwt", [81, 256, 256], F32R, kind="ExternalInput")
    pcb = nc.dram_tensor("pcb", [256], F32, kind="ExternalInput")
    w2n = nc.dram_tensor("w2n", [NS, HL], F32R, kind="ExternalInput")
    w2nt = nc.dram_tensor("w2nt", [HL, NS], F32R, kind="ExternalInput")
    eye64 = nc.dram_tensor("eye64", [BC, BC], F32R, kind="ExternalInput")
    vout = nc.dram_tensor("vout", [BC, HL], F32R, kind="ExternalOutput")

    pc_rd = nc.dram_tensor("pc_rd", [NS, BC], F32R)    # [r, b]

    with tile.TileContext(nc) as tc:
        with (
            tc.tile_pool(name="pers", bufs=1) as pers,
            tc.tile_pool(name="dram", bufs=1, space="DRAM") as dpool,
        ):
            w1t_sb = pers.tile([81, 256], F32R)
            nc.sync.dma_start(w1t_sb[:], w1t[:])
            b1_sb = pers.tile([128, 2], F32)
            nc.sync.dma_start(b1_sb[:], _r(b1[:], [[1, 128], [128, 2]]))
            pcb_sb = pers.tile([128, 2], F32)
            nc.sync.dma_start(pcb_sb[:], _r(pcb[:], [[1, 128], [128, 2]]))
            ones128 = pers.tile([128, 1], F32)
            nc.gpsimd.memset(ones128[:], 1.0)
            ones1 = pers.tile([1, 128], F32)
            nc.gpsimd.memset(ones1[:], 1.0)
            b9 = pers.tile([128, 9], F32)
            eye_sb = pers.tile([BC, BC], F32R)
            nc.sync.dma_start(eye_sb[:], eye64[:])

            # ---------------- conv phase ----------------
            with (
                tc.tile_pool(name="convsb", bufs=1) as csb,
                tc.tile_pool(name="pwp", bufs=3) as pwp,
                tc.tile_pool(name="ps1p", bufs=2, space="PSUM") as ps1p,
                tc.tile_pool(name="ps2p", bufs=2, space="PSUM") as ps2p,
            ):
                acc0 = csb.tile([128, BC * 36], F32)
                acc1 = csb.tile([128, BC * 36], F32)
                accs = [acc0, acc1]
                for ci_blk in range(2):
                    h1 = csb.tile([128, BC * 400], F32R, tag="h1")
                    hp = h1.ap[0][0]
                    for i in range(BC):
                        pw = pwp.tile([81, 560], F32R, tag="pw")
                        nc.sync.dma_start(
                            pw[:],
                            AP(xs[:].tensor, i * 800, [[28, 9], [1, 9], [1, 560]]),
                        )
                        ps1 = ps1p.tile([128, 400], F32, tag="ps1")
                        rhs = _r(pw, [[pw.ap[0][0], 81], [28, 20], [1, 20]])
                        out4 = _r(ps1, [[ps1.ap[0][0], 128], [20, 20], [1, 20]])
                        nc.tensor.matmul(
                            out4,
                            w1t_sb[:, ci_blk * 128:(ci_blk + 1) * 128],
                            rhs,
                            start=True, stop=True,
                        )
                        nc.scalar.activation(
                            h1[:, i * 400:(i + 1) * 400], ps1[:], AF.Relu,
                            bias=b1_sb[:, ci_blk:ci_blk + 1],
                        )
                    for co_blk in range(2):
                        w2c = csb.tile([128, 81 * 128], F32R, tag="w2c")
                        nc.sync.dma_start(
                            w2c[:],
                            AP(pcwt[:].tensor,
                               ci_blk * 128 * 256 + co_blk * 128,
                               [[256, 128], [256 * 256, 81], [1, 128]]),
                        )
                        for (g0, nb) in GROUPS:
                            ps2 = ps2p.tile([128, 504], F32, tag="ps2")
                            pstep = ps2.ap[0][0]
                            for kk in range(81):
                                ky, kx = divmod(kk, 9)
                                rhs = AP(h1.tensor,
                                         h1.offset + g0 * 400 + ky * 20 + kx,
                                         [[hp, 128], [400, nb], [40, 6], [2, 6]])
                                out4 = _r(ps2, [[pstep, 128], [36, nb], [6, 6], [1, 6]])
                                nc.tensor.matmul(
                                    out4,
                                    w2c[:, kk * 128:(kk + 1) * 128],
                                    rhs,
                                    start=(kk == 0), stop=(kk == 80),
                                )
                            dst = accs[co_blk][:, g0 * 36:(g0 + nb) * 36]
                            if ci_blk == 0:
                                nc.scalar.copy(dst, ps2[:, :nb * 36])
                            else:
                                nc.vector.tensor_tensor(dst, dst, ps2[:, :nb * 36], AL.add)
                # bias + relu -> pc2 (pix-major) -> pc_rd[r, b] in DRAM
                for co_blk in range(2):
                    pc2 = csb.tile([128, BC * 36], F32R, tag="pc2")
                    p2 = pc2.ap[0][0]
                    nc.scalar.activation(
                        _r(pc2, [[p2, 128], [1, BC], [BC, 36]]),
                        _r(accs[co_blk], [[accs[co_blk].ap[0][0], 128], [36, BC], [1, 36]]),
                        AF.Relu,
                        bias=pcb_sb[:, co_blk:co_blk + 1],
                    )
                    nc.sync.dma_start(
                        AP(pc_rd[:].tensor, co_blk * 128 * 36 * BC,
                           [[36 * BC, 128], [BC, 36], [1, BC]]),
                        _r(pc2, [[p2, 128], [BC, 36], [1, BC]]),
                    )

            # ---------------- routing phase ----------------
            with (
                tc.tile_pool(name="rsb", bufs=1) as rsb,
                tc.tile_pool(name="rnd", bufs=2) as rnd,
                tc.tile_pool(name="sps", bufs=1, space="PSUM") as sps,
                tc.tile_pool(name="gps", bufs=4, space="PSUM") as gps,
                tc.tile_pool(name="zps", bufs=1, space="PSUM") as zps,
            ):
                w2sb = rsb.tile([128, NT * HL], F32R)
                nc.sync.dma_start(
                    w2sb[:],
                    AP(w2n[:].tensor, 0, [[HL, 128], [128 * HL, NT], [1, HL]]),
                )
                # W2n^T in two hl-chunks: (128, NT*128) + (32, NT*128)
                w2nt_a = rsb.tile([128, NT * 128], F32R)
                nc.sync.dma_start(
                    w2nt_a[:],
                    AP(w2nt[:].tensor, 0, [[NS, 128], [128, NT], [1, 128]]),
                )
                w2nt_b = rsb.tile([32, NT * 128], F32R)
                nc.sync.dma_start(
                    w2nt_b[:],
                    AP(w2nt[:].tensor, 128 * NS, [[NS, 32], [128, NT], [1, 128]]),
                )
                xrT = rsb.tile([128, NT * BC], F32R)
                nc.sync.dma_start(
                    xrT[:],
                    AP(pc_rd[:].tensor, 0, [[BC, 128], [128 * BC, NT], [1, BC]]),
                )
                p_all = rsb.tile([128, NT * BC], F32)
                prod = rsb.tile([128, (NT // 2) * BC], F32)

                def s_matmul():
                    s_ps = sps.tile([BC, HL], F32, tag="s_ps")
                    for t in range(NT):
                        nc.tensor.matmul(
                            s_ps[:],
                            xrT[:, t * BC:(t + 1) * BC],
                            w2sb[:, t * HL:(t + 1) * HL],
                            start=(t == 0), stop=(t == NT - 1),
                        )
                    return s_ps

                def squash(s_sb):
                    sq = rnd.tile([BC, HL], F32, tag="sq")
                    nc.scalar.square(sq[:], s_sb[:])
                    n2 = rnd.tile([BC, 16], F32, tag="n2")
                    nc.vector.tensor_reduce(
                        n2[:].rearrange("a b -> a b ()"),
                        _r(sq, [[sq.ap[0][0], BC], [1, 16], [16, 10]]),
                        AX.X, AL.add,
                    )
                    rt = rnd.tile([BC, 16], F32, tag="rt")
                    nc.scalar.sqrt(rt[:], n2[:])
                    n2p1 = rnd.tile([BC, 16], F32, tag="n2p1")
                    nc.vector.tensor_scalar_add(n2p1[:], n2[:], 1.0)
                    rcp = rnd.tile([BC, 16], F32, tag="rcp")
                    nc.vector.reciprocal(rcp[:], n2p1[:])
                    f = rnd.tile([BC, 16], F32, tag="f")
                    nc.vector.tensor_tensor(f[:], rt[:], rcp[:], AL.mult)
                    v_sb = rnd.tile([BC, HL], F32R, tag="v_sb")
                    nc.vector.tensor_tensor(
                        _r(v_sb, [[v_sb.ap[0][0], BC], [16, 10], [1, 16]]),
                        _r(s_sb, [[s_sb.ap[0][0], BC], [16, 10], [1, 16]]),
                        _r(f, [[f.ap[0][0], BC], [0, 10], [1, 16]]),
                        AL.mult,
                    )
                    return v_sb

                def p_delta_update(v_sb, rnd_idx, rce9):
                    """delta via P[r,b] = sum_hl W2n[r,hl] v[b,hl] (PE), then
                    D[r] = sum_b xrT[r,b]*P[r,b] (DVE). If xrT is c-scaled,
                    divide delta9 by ce9 (rce9 ap) to undo."""
                    vt_ps = gps.tile([128, BC], F32R, tag="vt_ps", bufs=1)
                    nc.tensor.transpose(vt_ps[:], v_sb[:, 0:128], eye_sb[:])
                    vt_a = rnd.tile([128, BC], F32R, tag="vt_a")
                    nc.scalar.copy(vt_a[:], vt_ps[:])
                    vtb_ps = gps.tile([32, BC], F32R, tag="vtb_ps", bufs=1)
                    nc.tensor.transpose(vtb_ps[:], v_sb[:, 128:160], eye_sb[:])
                    vt_b = rnd.tile([32, BC], F32R, tag="vt_b")
                    nc.scalar.copy(vt_b[:], vtb_ps[:])
                    for t in range(NT):
                        p_ps = gps.tile([128, BC], F32, tag="p_ps", bufs=3)
                        nc.tensor.matmul(
                            p_ps[:],
                            w2nt_a[:, t * 128:(t + 1) * 128],
                            vt_a[:],
                            start=True, stop=False,
                        )
                        nc.tensor.matmul(
                            p_ps[:],
                            w2nt_b[:, t * 128:(t + 1) * 128],
                            vt_b[:],
                            start=False, stop=True,
                        )
                        nc.scalar.copy(p_all[:, t * BC:(t + 1) * BC], p_ps[:])
                    D = rnd.tile([128, NT], F32, tag="D")
                    half = (NT // 2) * BC
                    for hx in range(2):
                        nc.vector.tensor_tensor(
                            prod[:],
                            xrT[:, hx * half:(hx + 1) * half].bitcast(F32),
                            p_all[:, hx * half:(hx + 1) * half],
                            AL.mult,
                        )
                        nc.vector.tensor_reduce(
                            D[:, hx * (NT // 2):(hx + 1) * (NT // 2)]
                            .rearrange("a b -> a b ()"),
                            _r(prod, [[prod.ap[0][0], 128], [BC, NT // 2], [1, BC]]),
                            AX.X, AL.add,
                        )
                    delta9 = rnd.tile([128, 9], F32, tag="delta9")
                    nc.vector.tensor_reduce(
                        delta9[:].rearrange("a b -> a b ()"),
                        _r(D, [[D.ap[0][0], 128], [1, 9], [9, 8]]),
                        AX.X, AL.add,
                    )
                    if rce9 is not None:
                        nc.vector.tensor_tensor(delta9[:], delta9[:], rce9[:], AL.mult)
                    cin = dpool.tile([128, 9], F32, name=f"cin{rnd_idx}")
                    cout = dpool.tile([NCORES * 128, 9], F32, name=f"cout{rnd_idx}",
                                      addr_space="Shared")
                    nc.gpsimd.dma_start(cin[:], delta9[:])
                    nc.gpsimd.collective_compute(
                        "AllGather", AL.bypass,
                        replica_groups=[list(range(NCORES))],
                        ins=[cin.opt()], outs=[cout.opt()],
                    )
                    agg = rnd.tile([128, 8 * 9], F32, tag="agg")
                    nc.gpsimd.dma_start(
                        agg[:],
                        AP(cout.tensor, cout.offset, [[9, 128], [1, 9], [128 * 9, 8]]),
                    )
                    dsum = rnd.tile([128, 9], F32, tag="dsum")
                    nc.vector.tensor_reduce(
                        dsum[:].rearrange("a b -> a b ()"),
                        _r(agg, [[agg.ap[0][0], 128], [1, 9], [9, 8]]),
                        AX.X, AL.add,
                    )
                    if rnd_idx == 0:
                        nc.scalar.mul(b9[:], dsum[:], ROUTE_SCALE)
                    else:
                        sc = rnd.tile([128, 9], F32, tag="sc")
                        nc.scalar.mul(sc[:], dsum[:], ROUTE_SCALE)
                        nc.vector.tensor_tensor(b9[:], b9[:], sc[:], AL.add)

                def softmax_ce9():
                    """ce9[p,j] = softmax(b9)[n=j*128+p], F32R (128,9)."""
                    e9 = rnd.tile([128, 9], F32, tag="e9")
                    nc.scalar.activation(e9[:], b9[:], AF.Exp)
                    rs9 = rnd.tile([128, 1], F32, tag="rs9")
                    nc.vector.tensor_reduce(
                        rs9[:].rearrange("a b -> a b ()"), e9[:], AX.X, AL.add)
                    z_ps = zps.tile([1, 1], F32, tag="z_ps")
                    nc.tensor.matmul(z_ps[:], ones128[:], rs9[:], start=True, stop=True)
                    z_sb = rnd.tile([1, 1], F32, tag="z_sb")
                    nc.scalar.copy(z_sb[:], z_ps[:])
                    zb_ps = zps.tile([128, 1], F32, tag="zb_ps")
                    nc.tensor.matmul(zb_ps[:], ones1[:], z_sb[:], start=True, stop=True)
                    rz = rnd.tile([128, 1], F32, tag="rz")
                    nc.vector.reciprocal(rz[:], zb_ps[:])
                    ce9 = rnd.tile([128, 9], F32R, tag="ce9")
                    nc.vector.tensor_scalar_mul(ce9[:], e9[:], rz[:])
                    return ce9

                def scale_xrT(m9):
                    """xrT[p, (q,j,b)] *= m9[p, j] in place."""
                    nc.vector.tensor_tensor(
                        _r(xrT, [[xrT.ap[0][0], 128], [9 * BC, 8], [BC, 9], [1, BC]]),
                        _r(xrT, [[xrT.ap[0][0], 128], [9 * BC, 8], [BC, 9], [1, BC]]),
                        _r(m9, [[m9.ap[0][0], 128], [0, 8], [1, 9], [0, BC]]),
                        AL.mult,
                    )

                # ---- round 1 (c uniform; xrT unscaled) ----
                s_ps = s_matmul()
                s_sb = rnd.tile([BC, HL], F32, tag="s_sb")
                nc.scalar.mul(s_sb[:], s_ps[:], 1.0 / 1152.0)
                v_sb = squash(s_sb)
                p_delta_update(v_sb, 0, None)
                # ---- round 2 ----
                ce9_2 = softmax_ce9()
                scale_xrT(ce9_2)
                rce9 = rnd.tile([128, 9], F32, tag="rce9")
                nc.vector.reciprocal(rce9[:], ce9_2[:].bitcast(F32))
                s_ps = s_matmul()
                s_sb = rnd.tile([BC, HL], F32, tag="s_sb")
                nc.scalar.copy(s_sb[:], s_ps[:])
                v_sb = squash(s_sb)
                p_delta_update(v_sb, 1, rce9)
                # ---- round 3 (b update dead) ----
                ce9_3 = softmax_ce9()
                ratio9 = rnd.tile([128, 9], F32R, tag="ratio9")
                nc.vector.tensor_tensor(ratio9[:], ce9_3[:].bitcast(F32), rce9[:], AL.mult)
                scale_xrT(ratio9)
                s_ps = s_matmul()
                s_sb = rnd.tile([BC, HL], F32, tag="s_sb")
                nc.scalar.copy(s_sb[:], s_ps[:])
                v_sb = squash(s_sb)
                nc.sync.dma_start(vout[:], v_sb[:])

    return nc


_NC_CACHE = None


def _get_nc():
    global _NC_CACHE
    if _NC_CACHE is None:
        nc = build_nc()
        split_waits(nc)
        _NC_CACHE = nc
    return _NC_CACHE


def prepare_inputs(x, conv1_w, conv1_b, pc_w, pc_b, W):
    x = np.asarray(x, np.float32)
    xs = np.zeros((B, 800), np.float32)
    xs[:, :784] = x.reshape(B, 784)
    w1t = np.ascontiguousarray(np.asarray(conv1_w, np.float32).reshape(256, 81).T)
    b1 = np.ascontiguousarray(np.asarray(conv1_b, np.float32))
    pcwt = np.ascontiguousarray(
        np.asarray(pc_w, np.float32).reshape(256, 256, 81).transpose(2, 1, 0))
    pcb = np.ascontiguousarray(np.asarray(pc_b, np.float32).reshape(256))
    w2n = np.ascontiguousarray(
        np.asarray(W, np.float32).transpose(3, 0, 1, 2).reshape(NS, HL))
    w2nt = np.ascontiguousarray(w2n.T)
    eye64 = np.eye(BC, dtype=np.float32)
    in_maps = []
    for c in range(NCORES):
        in_maps.append({
            "xs": np.ascontiguousarray(xs[c * BC:(c + 1) * BC]),
            "w1t": w1t, "b1": b1, "pcwt": pcwt, "pcb": pcb, "w2n": w2n,
            "w2nt": w2nt, "eye64": eye64,
        })
    return in_maps


def kernel(x, conv1_w, conv1_b, pc_w, pc_b, W, _trace=False, _trace_kwargs=None):
    nc = _get_nc()
    in_maps = prepare_inputs(x, conv1_w, conv1_b, pc_w, pc_b, W)
    res = run_bass_kernel_spmd(
        nc, in_maps, list(range(NCORES)),
        trace=_trace, **(_trace_kwargs or {}),
    )
    v = np.concatenate([np.asarray(res.results[c]["vout"]) for c in range(NCORES)], 0)
    out = v.reshape(B, 1, 1, 10, 16).astype(np.float32)
    if _trace:
        return out, res
    return out



# revision 9
# speedup vs baseline: 1.5309x; 1.5309x over previous
"""CapsNet forward kernel for Trainium2, 8-core data-parallel.

Strategy (per spec sharding_hint): batch (512) split across 8 cores (64 each);
all params replicated. Routing logits b are a batch-mean -> AllReduce of
per-core partial deltas (1152 floats) per routing round (rounds 1,2 only;
round 3's b update is dead in the reference).

v2: all big matmuls in fp16 (1 cycle/row on the PE vs 2 for fp32-HIGH),
batched conv1 im2col DMA, conv2 accumulates both ci-blocks in PSUM,
routing weights prefetched during conv, AllGather+local-sum replaced by
AllReduce, and a warm-up collective mid-conv absorbs inter-core skew so
the first real AllReduce doesn't eat ~40us of stall.

Math restructuring (keeps exact semantics, avoids materializing u):
  r := s*1152 + n  (s=caps idx, n=(c32,oy,ox))  == co*36 + pix  with co=s*32+c32
  xr2[b, r]   = primary-caps output (relu), flattened
  W2n[r, hl]  = W.transpose(3,0,1,2).reshape(9216,160)
  s[b,hl]  = sum_r c[n(r)] * W2n[r,hl] * xr2[b,r]        (matmul, K=9216)
  v        = squash_dim1(s)
  P[r,b]   = sum_hl W2n[r,hl] * v[b,hl]                  (matmul)
  delta[n] = 1/(B*160) * sum_s sum_b xr2[b,r]*P[r,b]     (DVE TT+reduce)
Convs are PE matmuls: conv1 via in-SBUF "wide patch" im2col (K=81),
primary-caps conv via 81 shifted-window matmuls accumulated in PSUM (K=256,
both 128-chunks accumulated in the same PSUM bank).
"""

import numpy as np

import concourse.bass as bass
import concourse.mybir as mybir
import concourse.tile as tile
from concourse.ap import AP
from concourse.bass_utils import run_bass_kernel_spmd

F32 = mybir.dt.float32
F16 = mybir.dt.float16
AL = mybir.AluOpType
AF = mybir.ActivationFunctionType
AX = mybir.AxisListType

NCORES = 8
B = 512
BC = B // NCORES           # 64 images per core
MAX_WAITS = 1              # walrus on this path allows 1 sync wait per inst
HL = 160                   # 10 classes x 16 pose
NS = 9216                  # 1152 caps x 8
NT = NS // 128             # 72 K-tiles
GROUPS = [(0, 14), (14, 14), (28, 14), (42, 14), (56, 8)]  # conv2 image groups
ROUTE_SCALE = 1.0 / (B * HL)
CHUNK = 16                 # conv1 images per im2col DMA chunk


def _r(t, dims):
    """Raw AP on tile/ap t with explicit [step, count] dims (elements)."""
    return AP(t.tensor, t.offset, dims)


def split_waits(nc, max_waits=MAX_WAITS):
    """This walrus build rejects >max_waits sync waits per instruction; move
    excess waits onto same-engine NoOps inserted immediately before."""
    for f in nc.m.functions:
        for blk in f.blocks:
            out = []
            for ins in blk.instructions:
                si = ins.sync_info
                if si is not None and si.on_wait and len(si.on_wait) > max_waits:
                    waits = list(si.on_wait)
                    k = 0
                    while len(waits) > max_waits:
                        chunk, waits = waits[:max_waits], waits[max_waits:]
                        nop = mybir.InstNoOp(name=f"{ins.name}-ws{k}", ins=[], outs=[])
                        nop.engine = ins.engine
                        nop.sync_info = mybir.SyncInfo(on_wait=chunk, on_update=[])
                        out.append(nop)
                        k += 1
                    ins.sync_info = mybir.SyncInfo(
                        on_wait=waits, on_update=list(si.on_update or []))
                out.append(ins)
            blk.instructions = out


def build_nc():
    nc = bass.Bass(num_devices=NCORES)

    xpatch = nc.dram_tensor("xpatch", [81, BC * 560], F16, kind="ExternalInput")
    w1t = nc.dram_tensor("w1t", [81, 256], F16, kind="ExternalInput")
    b1 = nc.dram_tensor("b1", [256], F32, kind="ExternalInput")
    pcwt = nc.dram_tensor("pcwt", [81, 256, 256], F16, kind="ExternalInput")
    pcb = nc.dram_tensor("pcb", [256], F32, kind="ExternalInput")
    w2n = nc.dram_tensor("w2n", [NS, HL], F16, kind="ExternalInput")
    w2nt = nc.dram_tensor("w2nt", [HL, NS], F16, kind="ExternalInput")
    eye64 = nc.dram_tensor("eye64", [BC, BC], F16, kind="ExternalInput")
    vout = nc.dram_tensor("vout", [BC, HL], F32, kind="ExternalOutput")

    pc_rd = nc.dram_tensor("pc_rd", [NS, BC], F16)    # [r, b]

    with tile.TileContext(nc) as tc:
        with (
            tc.tile_pool(name="pers", bufs=1) as pers,
            tc.tile_pool(name="dram", bufs=1, space="DRAM") as dpool,
        ):
            w1t_sb = pers.tile([81, 256], F16)
            nc.sync.dma_start(w1t_sb[:], w1t[:])
            b1_sb = pers.tile([128, 2], F32)
            nc.sync.dma_start(b1_sb[:], _r(b1[:], [[1, 128], [128, 2]]))
            pcb_sb = pers.tile([128, 2], F32)
            nc.sync.dma_start(pcb_sb[:], _r(pcb[:], [[1, 128], [128, 2]]))
            ones128 = pers.tile([128, 1], F32)
            nc.gpsimd.memset(ones128[:], 1.0)
            ones1 = pers.tile([1, 128], F32)
            nc.gpsimd.memset(ones1[:], 1.0)
            b9 = pers.tile([128, 9], F32)
            eye_sb = pers.tile([BC, BC], F16)
            nc.sync.dma_start(eye_sb[:], eye64[:])
            # routing s-weights: needed right at routing start -> prefetch now
            w2sb = pers.tile([128, NT * HL], F16)

            # ---------------- conv phase ----------------
            with tc.tile_pool(name="h1p", bufs=1) as h1p:
                h1s = [h1p.tile([128, BC * 400], F16, tag=f"h1_{ci}",
                                name=f"h1_{ci}")
                       for ci in range(2)]
                with (
                    tc.tile_pool(name="pwp", bufs=2) as pwp,
                    tc.tile_pool(name="ps1p", bufs=4, space="PSUM") as ps1p,
                ):
                    NCH = BC // CHUNK
                    pas = []

                    def load_chunk(k, eng):
                        pa = pwp.tile([81, CHUNK * 560], F16, tag="pa")
                        eng.dma_start(
                            pa[:],
                            AP(xpatch[:].tensor, k * CHUNK * 560,
                               [[BC * 560, 81], [1, CHUNK * 560]]),
                        )
                        return pa

                    pas.append(load_chunk(0, nc.sync))
                    pas.append(load_chunk(1, nc.scalar))
                    # prefetch routing weights behind the first chunks
                    nc.sync.dma_start(
                        w2sb[:],
                        AP(w2n[:].tensor, 0, [[HL, 128], [128 * HL, NT], [1, HL]]),
                    )
                    for k in range(NCH):
                        pa = pas[k]
                        pstep = pa.ap[0][0]
                        for li in range(CHUNK):
                            gi = k * CHUNK + li
                            for ci in range(2):
                                ps = ps1p.tile([128, 400], F32, tag="ps1")
                                rhs = AP(pa.tensor, pa.offset + li * 560,
                                         [[pstep, 81], [28, 20], [1, 20]])
                                nc.tensor.matmul(
                                    ps[:],
                                    w1t_sb[:, ci * 128:(ci + 1) * 128],
                                    rhs,
                                    start=True, stop=True,
                                )
                                dst = h1s[ci][:, gi * 400:(gi + 1) * 400]
                                if ci == 0:
                                    nc.scalar.activation(
                                        dst, ps[:], AF.Relu,
                                        bias=b1_sb[:, ci:ci + 1],
                                    )
                                else:
                                    nc.vector.tensor_scalar(
                                        dst, ps[:], b1_sb[:, ci:ci + 1], 0.0,
                                        AL.add, AL.max,
                                    )
                            if li == 0 and k + 2 < NCH:
                                pas.append(load_chunk(
                                    k + 2, nc.sync if k % 2 == 0 else nc.scalar))

                # ---- conv2: 81 shifted matmuls, K=256 via 2 PSUM-accumulated
                # 128-chunks ----
                with (
                    tc.tile_pool(name="w2cp", bufs=2) as w2cp,
                    tc.tile_pool(name="ps2p", bufs=5, space="PSUM") as ps2p,
                    tc.tile_pool(name="pc2p", bufs=2) as pc2p,
                ):
                    def load_w2c(co, ci, eng):
                        t = w2cp.tile([128, 81 * 128], F16, tag="w2c")
                        eng.dma_start(
                            t[:],
                            AP(pcwt[:].tensor, ci * 128 * 256 + co * 128,
                               [[256, 128], [256 * 256, 81], [1, 128]]),
                        )
                        return t

                    w2c0 = [load_w2c(0, 0, nc.sync), load_w2c(0, 1, nc.scalar)]
                    for co_blk in range(2):
                        w2cs = w2c0 if co_blk == 0 else \
                            [load_w2c(1, 0, nc.sync), load_w2c(1, 1, nc.scalar)]
                        pc2 = pc2p.tile([128, BC * 36], F16, tag="pc2")
                        p2 = pc2.ap[0][0]
                        for (g0, nb) in GROUPS:
                            ps2 = ps2p.tile([128, 504], F32, tag="ps2")
                            pstep = ps2.ap[0][0]
                            out4 = _r(ps2, [[pstep, 128], [36, nb], [6, 6], [1, 6]])
                            for ci in range(2):
                                h1 = h1s[ci]
                                hp = h1.ap[0][0]
                                for kk in range(81):
                                    ky, kx = divmod(kk, 9)
                                    rhs = AP(h1.tensor,
                                             h1.offset + g0 * 400 + ky * 20 + kx,
                                             [[hp, 128], [400, nb], [40, 6], [2, 6]])
                                    nc.tensor.matmul(
                                        out4,
                                        w2cs[ci][:, kk * 128:(kk + 1) * 128],
                                        rhs,
                                        start=(ci == 0 and kk == 0),
                                        stop=(ci == 1 and kk == 80),
                                    )
                            # bias+relu, write pix-major (col = pix*BC + b)
                            nc.scalar.activation(
                                AP(pc2.tensor, pc2.offset + g0,
                                   [[p2, 128], [1, nb], [BC, 36]]),
                                _r(ps2, [[pstep, 128], [36, nb], [1, 36]]),
                                AF.Relu,
                                bias=pcb_sb[:, co_blk:co_blk + 1],
                            )
                        # pc2 -> pc_rd[r, b] in DRAM (r = co*36 + pix)
                        (nc.sync if co_blk == 0 else nc.scalar).dma_start(
                            AP(pc_rd[:].tensor, co_blk * 128 * 36 * BC,
                               [[36 * BC, 128], [BC, 36], [1, BC]]),
                            _r(pc2, [[p2, 128], [BC, 36], [1, BC]]),
                        )
                        if co_blk == 0:
                            # warm-up collective: absorbs inter-core skew off
                            # the critical path so the first real AllReduce is
                            # fast. Depends on pc2 (fires near conv end).
                            cinw = dpool.tile([128, 1], F32, name="cinw")
                            coutw = dpool.tile([128, 1], F32, name="coutw",
                                               addr_space="Shared")
                            nc.gpsimd.dma_start(cinw[:], pc2[:, 0:1])
                            nc.gpsimd.collective_compute(
                                "AllReduce", AL.add,
                                replica_groups=[list(range(NCORES))],
                                ins=[cinw.opt()], outs=[coutw.opt()],
                            )

            # ---------------- routing phase ----------------
            with (
                tc.tile_pool(name="rsb", bufs=1) as rsb,
                tc.tile_pool(name="rnd", bufs=2) as rnd,
                tc.tile_pool(name="sps", bufs=1, space="PSUM") as sps,
                tc.tile_pool(name="gps", bufs=4, space="PSUM") as gps,
                tc.tile_pool(name="zps", bufs=1, space="PSUM") as zps,
            ):
                # W2n^T in two hl-chunks: (128, NT*128) + (32, NT*128)
                w2nt_a = rsb.tile([128, NT * 128], F16)
                nc.sync.dma_start(
                    w2nt_a[:],
                    AP(w2nt[:].tensor, 0, [[NS, 128], [128, NT], [1, 128]]),
                )
                w2nt_b = rsb.tile([32, NT * 128], F16)
                nc.scalar.dma_start(
                    w2nt_b[:],
                    AP(w2nt[:].tensor, 128 * NS, [[NS, 32], [128, NT], [1, 128]]),
                )
                xrT = rsb.tile([128, NT * BC], F16)
                nc.sync.dma_start(
                    xrT[:],
                    AP(pc_rd[:].tensor, 0, [[BC, 128], [128 * BC, NT], [1, BC]]),
                )
                p_all = rsb.tile([128, NT * BC], F16)
                prod = rsb.tile([128, (NT // 2) * BC], F16)

                def s_matmul():
                    s_ps = sps.tile([BC, HL], F32, tag="s_ps")
                    for t in range(NT):
                        nc.tensor.matmul(
                            s_ps[:],
                            xrT[:, t * BC:(t + 1) * BC],
                            w2sb[:, t * HL:(t + 1) * HL],
                            start=(t == 0), stop=(t == NT - 1),
                        )
                    return s_ps

                def squash(s_sb, out_dtype):
                    sq = rnd.tile([BC, HL], F32, tag="sq")
                    nc.scalar.square(sq[:], s_sb[:])
                    n2 = rnd.tile([BC, 16], F32, tag="n2")
                    nc.vector.tensor_reduce(
                        n2[:].rearrange("a b -> a b ()"),
                        _r(sq, [[sq.ap[0][0], BC], [1, 16], [16, 10]]),
                        AX.X, AL.add,
                    )
                    rt = rnd.tile([BC, 16], F32, tag="rt")
                    nc.scalar.sqrt(rt[:], n2[:])
                    n2p1 = rnd.tile([BC, 16], F32, tag="n2p1")
                    nc.vector.tensor_scalar_add(n2p1[:], n2[:], 1.0)
                    rcp = rnd.tile([BC, 16], F32, tag="rcp")
                    nc.vector.reciprocal(rcp[:], n2p1[:])
                    f = rnd.tile([BC, 16], F32, tag="f")
                    nc.vector.tensor_tensor(f[:], rt[:], rcp[:], AL.mult)
                    v_sb = rnd.tile([BC, HL], out_dtype, tag="v_sb")
                    nc.vector.tensor_tensor(
                        _r(v_sb, [[v_sb.ap[0][0], BC], [16, 10], [1, 16]]),
                        _r(s_sb, [[s_sb.ap[0][0], BC], [16, 10], [1, 16]]),
                        _r(f, [[f.ap[0][0], BC], [0, 10], [1, 16]]),
                        AL.mult,
                    )
                    return v_sb

                def p_delta_update(v16, rnd_idx, rce9):
                    """delta via P[r,b] = sum_hl W2n[r,hl] v[b,hl] (PE), then
                    D[r] = sum_b xrT[r,b]*P[r,b] (DVE). If xrT is c-scaled,
                    divide delta9 by ce9 (rce9 ap) to undo."""
                    vt_ps = gps.tile([128, BC], F16, tag="vt_ps", bufs=1)
                    nc.tensor.transpose(vt_ps[:], v16[:, 0:128], eye_sb[:])
                    vt_a = rnd.tile([128, BC], F16, tag="vt_a")
                    nc.scalar.copy(vt_a[:], vt_ps[:])
                    vtb_ps = gps.tile([32, BC], F16, tag="vtb_ps", bufs=1)
                    nc.tensor.transpose(vtb_ps[:], v16[:, 128:160], eye_sb[:])
                    vt_b = rnd.tile([32, BC], F16, tag="vt_b")
                    nc.scalar.copy(vt_b[:], vtb_ps[:])
                    for t in range(NT):
                        p_ps = gps.tile([128, BC], F32, tag="p_ps", bufs=3)
                        nc.tensor.matmul(
                            p_ps[:],
                            w2nt_a[:, t * 128:(t + 1) * 128],
                            vt_a[:],
                            start=True, stop=False,
                        )
                        nc.tensor.matmul(
                            p_ps[:],
                            w2nt_b[:, t * 128:(t + 1) * 128],
                            vt_b[:],
                            start=False, stop=True,
                        )
                        nc.scalar.copy(p_all[:, t * BC:(t + 1) * BC], p_ps[:])
                    D = rnd.tile([128, NT], F32, tag="D")
                    half = (NT // 2) * BC
                    for hx in range(2):
                        nc.vector.tensor_tensor(
                            prod[:],
                            xrT[:, hx * half:(hx + 1) * half],
                            p_all[:, hx * half:(hx + 1) * half],
                            AL.mult,
                        )
                        nc.vector.tensor_reduce(
                            D[:, hx * (NT // 2):(hx + 1) * (NT // 2)]
                            .rearrange("a b -> a b ()"),
                            _r(prod, [[prod.ap[0][0], 128], [BC, NT // 2], [1, BC]]),
                            AX.X, AL.add,
                        )
                    delta9 = rnd.tile([128, 9], F32, tag="delta9")
                    nc.vector.tensor_reduce(
                        delta9[:].rearrange("a b -> a b ()"),
                        _r(D, [[D.ap[0][0], 128], [1, 9], [9, 8]]),
                        AX.X, AL.add,
                    )
                    if rce9 is not None:
                        nc.vector.tensor_tensor(delta9[:], delta9[:], rce9[:], AL.mult)
                    cin = dpool.tile([128, 9], F32, name=f"cin{rnd_idx}")
                    cout = dpool.tile([128, 9], F32, name=f"cout{rnd_idx}",
                                      addr_space="Shared")
                    nc.gpsimd.dma_start(cin[:], delta9[:])
                    nc.gpsimd.collective_compute(
                        "AllReduce", AL.add,
                        replica_groups=[list(range(NCORES))],
                        ins=[cin.opt()], outs=[cout.opt()],
                    )
                    dsum = rnd.tile([128, 9], F32, tag="dsum")
                    nc.gpsimd.dma_start(dsum[:], cout[:])
                    if rnd_idx == 0:
                        nc.scalar.mul(b9[:], dsum[:], ROUTE_SCALE)
                    else:
                        sc = rnd.tile([128, 9], F32, tag="sc")
                        nc.scalar.mul(sc[:], dsum[:], ROUTE_SCALE)
                        nc.vector.tensor_tensor(b9[:], b9[:], sc[:], AL.add)

                def softmax_ce9():
                    """ce9[p,j] = softmax(b9)[n=j*128+p], F32 (128,9)."""
                    e9 = rnd.tile([128, 9], F32, tag="e9")
                    nc.scalar.activation(e9[:], b9[:], AF.Exp)
                    rs9 = rnd.tile([128, 1], F32, tag="rs9")
                    nc.vector.tensor_reduce(
                        rs9[:].rearrange("a b -> a b ()"), e9[:], AX.X, AL.add)
                    z_ps = zps.tile([1, 1], F32, tag="z_ps")
                    nc.tensor.matmul(z_ps[:], ones128[:], rs9[:], start=True, stop=True)
                    z_sb = rnd.tile([1, 1], F32, tag="z_sb")
                    nc.scalar.copy(z_sb[:], z_ps[:])
                    zb_ps = zps.tile([128, 1], F32, tag="zb_ps")
                    nc.tensor.matmul(zb_ps[:], ones1[:], z_sb[:], start=True, stop=True)
                    rz = rnd.tile([128, 1], F32, tag="rz")
                    nc.vector.reciprocal(rz[:], zb_ps[:])
                    ce9 = rnd.tile([128, 9], F32, tag="ce9")
                    nc.vector.tensor_scalar_mul(ce9[:], e9[:], rz[:])
                    return ce9

                def scale_xrT(m9f32):
                    """xrT[p, (q,j,b)] *= m9[p, j] in place (m9 cast to F16)."""
                    m16 = rnd.tile([128, 9], F16, tag="m16")
                    nc.scalar.copy(m16[:], m9f32[:])
                    nc.vector.tensor_tensor(
                        _r(xrT, [[xrT.ap[0][0], 128], [9 * BC, 8], [BC, 9], [1, BC]]),
                        _r(xrT, [[xrT.ap[0][0], 128], [9 * BC, 8], [BC, 9], [1, BC]]),
                        _r(m16, [[m16.ap[0][0], 128], [0, 8], [1, 9], [0, BC]]),
                        AL.mult,
                    )

                # ---- round 1 (c uniform; xrT unscaled) ----
                s_ps = s_matmul()
                s_sb = rnd.tile([BC, HL], F32, tag="s_sb")
                nc.scalar.mul(s_sb[:], s_ps[:], 1.0 / 1152.0)
                v16 = squash(s_sb, F16)
                p_delta_update(v16, 0, None)
                # ---- round 2 ----
                ce9_2 = softmax_ce9()
                scale_xrT(ce9_2)
                rce9 = rnd.tile([128, 9], F32, tag="rce9")
                nc.vector.reciprocal(rce9[:], ce9_2[:])
                s_ps = s_matmul()
                s_sb = rnd.tile([BC, HL], F32, tag="s_sb")
                nc.scalar.copy(s_sb[:], s_ps[:])
                v16 = squash(s_sb, F16)
                p_delta_update(v16, 1, rce9)
                # ---- round 3 (b update dead) ----
                ce9_3 = softmax_ce9()
                ratio9 = rnd.tile([128, 9], F32, tag="ratio9")
                nc.vector.tensor_tensor(ratio9[:], ce9_3[:], rce9[:], AL.mult)
                scale_xrT(ratio9)
                s_ps = s_matmul()
                s_sb = rnd.tile([BC, HL], F32, tag="s_sb")
                nc.scalar.copy(s_sb[:], s_ps[:])
                v_sb = squash(s_sb, F32)
                nc.sync.dma_start(vout[:], v_sb[:])

    return nc


_NC_CACHE = None


def _get_nc():
    global _NC_CACHE
    if _NC_CACHE is None:
        nc = build_nc()
        split_waits(nc)
        _NC_CACHE = nc
    return _NC_CACHE


def prepare_inputs(x, conv1_w, conv1_b, pc_w, pc_b, W):
    x = np.asarray(x, np.float32)
    xf = np.zeros((B, 800), np.float16)
    xf[:, :784] = x.reshape(B, 784).astype(np.float16)
    # host-side im2col ("wide patch"): xp[i, (ky,kx), j] = xf[i, 28*ky+kx+j]
    xp = np.lib.stride_tricks.as_strided(
        xf, shape=(B, 9, 9, 560), strides=(1600, 56, 2, 2)).reshape(B, 81, 560)
    w1t = np.ascontiguousarray(
        np.asarray(conv1_w, np.float32).reshape(256, 81).T).astype(np.float16)
    b1 = np.ascontiguousarray(np.asarray(conv1_b, np.float32))
    pcwt = np.ascontiguousarray(
        np.asarray(pc_w, np.float32).reshape(256, 256, 81).transpose(2, 1, 0)
    ).astype(np.float16)
    pcb = np.ascontiguousarray(np.asarray(pc_b, np.float32).reshape(256))
    w2n = np.ascontiguousarray(
        np.asarray(W, np.float32).transpose(3, 0, 1, 2).reshape(NS, HL)
    ).astype(np.float16)
    w2nt = np.ascontiguousarray(w2n.T)
    eye64 = np.eye(BC, dtype=np.float16)
    in_maps = []
    for c in range(NCORES):
        in_maps.append({
            "xpatch": np.ascontiguousarray(
                xp[c * BC:(c + 1) * BC].transpose(1, 0, 2).reshape(81, BC * 560)),
            "w1t": w1t, "b1": b1, "pcwt": pcwt, "pcb": pcb, "w2n": w2n,
            "w2nt": w2nt, "eye64": eye64,
        })
    return in_maps


def kernel(x, conv1_w, conv1_b, pc_w, pc_b, W, _trace=False, _trace_kwargs=None):
    nc = _get_nc()
    in_maps = prepare_inputs(x, conv1_w, conv1_b, pc_w, pc_b, W)
    res = run_bass_kernel_spmd(
        nc, in_maps, list(range(NCORES)),
        trace=_trace, **(_trace_kwargs or {}),
    )
    v = np.concatenate([np.asarray(res.results[c]["vout"]) for c in range(NCORES)], 0)
    out = v.reshape(B, 1, 1, 10, 16).astype(np.float32)
    if _trace:
        return out, res
    return out
